# revision 14
# baseline (speedup 1.0000x reference)
"""GRU cell (single timestep) on 8 TRN2 NeuronCores, data-parallel over batch.

Contract: kernel(**inputs) takes FULL numpy inputs (as produced by the
problem's setup_inputs()) and returns the FULL (16384, 1024) float32 output.

Strategy:
  - Shard batch (16384) across 8 cores -> 2048 rows/core; 4 column-blocks
    of 512. Replicate weights.
  - All h-side matmuls, and the r/z-gate x-side, run in fp8 e4m3 with
    perf_mode=DoubleRow (2 k-tiles per MM -> ~1.9x PE rate); the hc-gate
    x-side stays bf16 (fp8 there pushes rel err to ~2e-2; this config
    measures ~1.5e-2 vs the 2e-2 gate). All weights pre-scaled by 1024
    (exact pow2 shift for bf16; lifts fp8 weights out of the subnormal
    range); the activation applies scale=1/1024 before bias+nonlinearity.
  - Activations + elementwise combine in bf16 (DVE 2x packed mode);
    fp32 PSUM accumulation.
  - Head: each dma_start costs ~3.5us of fixed issue/complete latency and
    transfers finish roughly FIFO by issue order, so the t=0 critical set
    is packed into just three fat transfers (same k-chunk structure,
    concatenated along the free dim):
      rxc [128, 4, 2560] fp8 = x8(block0) | 1024*Wxr | 1024*Wxz
      rhc [128, 8, 2560] fp8 = h8(block0) | 1024*Whr | 1024*Whz
      bbc [128, 12, 512] bf16 = xb(block0) | hb(block0)
    A GpSimd fence gated on bbc (the last t=0 transfer) releases the hc
    weights; blocks 1-3 (host-packed fp8+bf16) fetch from the Scalar
    queue after the r activations. 20 dummy matmuls on a memset tile
    warm the PE clock (HAM 1.2->2.4GHz) while the first DMAs land.
"""

import sys

if "/opt/trn_rl_repo" not in sys.path:
    sys.path.insert(0, "/opt/trn_rl_repo")

import numpy as np
import ml_dtypes

import concourse.bass as bass
import concourse.tile as tile
from concourse import bacc, mybir
from concourse.bass_utils import run_bass_kernel_spmd

P = 128
NCORES = 8
BATCH = 16384
NB = BATCH // NCORES          # 2048 rows per core
IN = 512
HID = 1024
KX = IN // P                  # 4
KH = HID // P                 # 8
KA = KX + KH                  # 12 packed k-chunks (x then h)
M = HID // P                  # 8 output-feature chunks
BLK = 512                     # batch columns per compute block
NBLK = NB // BLK              # 4
WSCALE = 1024.0               # pow2 pre-scale on all weights
Z_X_FP8 = True                # z-gate x-side in fp8 DoubleRow
N_WARM = 50                   # dummy matmuls to warm the PE clock

F32 = mybir.dt.float32
BF16 = mybir.dt.bfloat16
FP8 = mybir.dt.float8e4

SIG = mybir.ActivationFunctionType.Sigmoid
TANH = mybir.ActivationFunctionType.Tanh
DR = mybir.MatmulPerfMode.DoubleRow

_CACHE = {}


def _build():
    nc = bacc.Bacc("TRN2", target_bir_lowering=False, debug=False, num_devices=NCORES)

    rxc = nc.dram_tensor("rxc", [P, KX, BLK + 2 * HID], FP8, kind="ExternalInput").ap()
    rhc = nc.dram_tensor("rhc", [P, KH, BLK + HID], FP8, kind="ExternalInput").ap()
    whz = nc.dram_tensor("whz", [P, KH, HID], FP8, kind="ExternalInput").ap()
    bbc = nc.dram_tensor("bbc", [P, KA, BLK], BF16, kind="ExternalInput").ap()
    xh8 = nc.dram_tensor("xh8", [NBLK, P, KA, BLK], FP8, kind="ExternalInput").ap()
    xhb = nc.dram_tensor("xhb", [NBLK, P, KA, BLK], BF16, kind="ExternalInput").ap()
    wxh = nc.dram_tensor("wxh", [P, KX, HID], BF16, kind="ExternalInput").ap()
    whh = nc.dram_tensor("whh", [P, KH, HID], FP8, kind="ExternalInput").ap()
    bias = nc.dram_tensor("bias", [P, 24], F32, kind="ExternalInput").ap()
    outT = nc.dram_tensor("outT", [NBLK, P, M, BLK], BF16, kind="ExternalOutput").ap()

    inv_s = 1.0 / WSCALE

    with tile.TileContext(nc) as tc:
        with (
            tc.tile_pool(name="wpool", bufs=1) as wpool,
            tc.tile_pool(name="x8pool", bufs=3) as x8pool,
            tc.tile_pool(name="h8pool", bufs=3) as h8pool,
            tc.tile_pool(name="xbpool", bufs=3) as xbpool,
            tc.tile_pool(name="hbpool", bufs=3) as hbpool,
            tc.tile_pool(name="rpool", bufs=2) as rpool,
            tc.tile_pool(name="rhpool", bufs=2) as rhpool,
            tc.tile_pool(name="zpool", bufs=2) as zpool,
            tc.tile_pool(name="hcpool", bufs=2) as hcpool,
            tc.tile_pool(name="opool", bufs=4) as opool,
            tc.tile_pool(name="obpool", bufs=2) as obpool,
            tc.tile_pool(name="psum", bufs=8, space=bass.MemorySpace.PSUM) as psum,
        ):
            b_s = wpool.tile([P, 24], F32)
            rxc_s = wpool.tile([P, KX, BLK + 2 * HID], FP8)
            rhc_s = wpool.tile([P, KH, BLK + HID], FP8)
            whz_s = wpool.tile([P, KH, HID], FP8)
            bbc_s = wpool.tile([P, KA, BLK], BF16)
            wxh_s = wpool.tile([P, KX, HID], BF16)
            whh_s = wpool.tile([P, KH, HID], FP8)
            dummy = wpool.tile([P, BLK], BF16)
            fence = wpool.tile([P, 2], BF16)

            wxr_v = rxc_s[:, :, BLK : BLK + HID]
            wxz_v = rxc_s[:, :, BLK + HID : BLK + 2 * HID]
            whr_v = rhc_s[:, :, BLK : BLK + HID]
            whz_v = whz_s

            x8 = [None] * NBLK   # [P, 4, BLK] fp8
            h8 = [None] * NBLK   # [P, 8, BLK] fp8
            xb = [None] * NBLK   # [P, 4, BLK] bf16
            hb = [None] * NBLK   # [P, 8, BLK] bf16
            x8[0] = rxc_s[:, :, 0:BLK]
            h8[0] = rhc_s[:, :, 0:BLK]
            xb[0] = bbc_s[:, 0:KX, :]
            hb[0] = bbc_s[:, KX:KA, :]

            def fetch_block(blk, eng):
                x8[blk] = x8pool.tile([P, KX, BLK], FP8, tag="x8", name="x8")
                eng.dma_start(x8[blk][:], xh8[blk, :, 0:KX, :])
                h8[blk] = h8pool.tile([P, KH, BLK], FP8, tag="h8", name="h8")
                eng.dma_start(h8[blk][:], xh8[blk, :, KX:KA, :])
                hb[blk] = hbpool.tile([P, KH, BLK], BF16, tag="hb", name="hb")
                eng.dma_start(hb[blk][:], xhb[blk, :, KX:KA, :])
                xb[blk] = xbpool.tile([P, KX, BLK], BF16, tag="xb", name="xb")
                eng.dma_start(xb[blk][:], xhb[blk, :, 0:KX, :])

            # PE warmup: dummy matmuls on a memset tile while DMAs land.
            nc.vector.memset(dummy[:], 0.0)
            warm_ps = psum.tile([P, BLK], F32, tag="ps", name="ps")
            for _ in range(N_WARM):
                nc.tensor.matmul(
                    warm_ps[:], dummy[:, 0:P], dummy[:], start=True, stop=True,
                )

            # t=0 critical set: three fat transfers, FIFO priority order.
            nc.sync.dma_start(rxc_s[:], rxc[:])
            nc.sync.dma_start(rhc_s[:], rhc[:])
            nc.sync.dma_start(b_s[:], bias[:])
            nc.sync.dma_start(whz_s[:], whz[:])
            nc.sync.dma_start(bbc_s[:], bbc[:])

            # hc weights, gated on the LAST t=0 transfer so they can't
            # steal bandwidth from the critical set.
            nc.gpsimd.tensor_copy(fence[:], bbc_s[:, 0, 0:2])
            nc.gpsimd.dma_start(wxh_s[:], wxh[:])
            nc.gpsimd.dma_start(whh_s[:], whh[:])

            def x_mms(ps, wx, m, blk, fp8_x):
                mo = bass.ts(m, P)
                if fp8_x:
                    for j in range(KX // 2):
                        nc.tensor.matmul(
                            ps[:], wx[:, 2 * j : 2 * j + 2, mo],
                            x8[blk][:, 2 * j : 2 * j + 2, :],
                            start=(j == 0), stop=False, perf_mode=DR,
                        )
                else:
                    for k in range(KX):
                        nc.tensor.matmul(
                            ps[:], wx[:, k, mo], xb[blk][:, k, :],
                            start=(k == 0), stop=False,
                        )

            def h_mms(ps, wh, m, rhs8):
                mo = bass.ts(m, P)
                for j in range(KH // 2):
                    nc.tensor.matmul(
                        ps[:], wh[:, 2 * j : 2 * j + 2, mo],
                        rhs8[:, 2 * j : 2 * j + 2, :],
                        start=False, stop=(j == KH // 2 - 1),
                        perf_mode=DR,
                    )

            for blk in range(NBLK):
                split = blk == 0  # emit x-groups before h-groups on block 0

                # ---- R phase: r = sigmoid(x@Wxr + bxr + h@Whr); rh8 = fp8(r*h)
                rh8 = rhpool.tile([P, KH, BLK], FP8, tag="rh8")
                ps_r = []
                if split:
                    for m in range(M - 1):
                        ps = psum.tile([P, BLK], F32, tag="ps", name="ps")
                        ps_r.append(ps)
                        x_mms(ps, wxr_v, m, blk, True)
                    # filler on the (free) warm bank: keeps the HAM clock
                    # hot through the rhc-arrival wait so the h-side MMs
                    # run at 2.4GHz instead of re-throttled 1.2GHz.
                    for _ in range(16):
                        nc.tensor.matmul(
                            warm_ps[:], dummy[:, 0:P], dummy[:],
                            start=True, stop=True,
                        )
                    for m in range(M - 1):
                        h_mms(ps_r[m], whr_v, m, h8[blk])
                    ps = psum.tile([P, BLK], F32, tag="ps", name="ps")
                    ps_r.append(ps)
                    x_mms(ps, wxr_v, M - 1, blk, True)
                    h_mms(ps, whr_v, M - 1, h8[blk])
                else:
                    for m in range(M):
                        ps = psum.tile([P, BLK], F32, tag="ps", name="ps")
                        ps_r.append(ps)
                        x_mms(ps, wxr_v, m, blk, True)
                        h_mms(ps, whr_v, m, h8[blk])
                for m in range(M):
                    rt = rpool.tile([P, BLK], BF16, tag="rt")
                    nc.scalar.activation(
                        rt[:], ps_r[m][:], SIG,
                        bias=b_s[:, m : m + 1], scale=inv_s,
                    )
                    eng = nc.vector if m % 2 == 0 else nc.gpsimd
                    eng.tensor_mul(rh8[:, m, :], rt[:], hb[blk][:, m, :])

                if blk == 0:
                    fetch_block(1, nc.scalar)

                # ---- Z phase: z = sigmoid(x@Wxz + bxz + h@Whz)
                zf = zpool.tile([P, M, BLK], BF16, tag="zf")
                ps_z = []
                for m in range(M):
                    ps = psum.tile([P, BLK], F32, tag="ps", name="ps")
                    ps_z.append(ps)
                    x_mms(ps, wxz_v, m, blk, Z_X_FP8)
                    if not split:
                        h_mms(ps, whz_v, m, h8[blk])
                if split:
                    for m in range(M):
                        h_mms(ps_z[m], whz_v, m, h8[blk])
                for m in range(M):
                    nc.scalar.activation(
                        zf[:, m, :], ps_z[m][:], SIG,
                        bias=b_s[:, 8 + m : 9 + m], scale=inv_s,
                    )

                if blk + 2 < NBLK:
                    fetch_block(blk + 2, nc.scalar)

                # ---- HC phase: hc = tanh(x@Wxh + bxh + rh@Whh)
                #      out = hc + z*(h - hc)
                ob = obpool.tile([P, M, BLK], BF16, tag="ob")
                for m in range(M):
                    ps = psum.tile([P, BLK], F32, tag="ps", name="ps")
                    x_mms(ps, wxh_s, m, blk, False)
                    h_mms(ps, whh_s, m, rh8)
                    hct = hcpool.tile([P, BLK], BF16, tag="hct")
                    nc.scalar.activation(
                        hct[:], ps[:], TANH,
                        bias=b_s[:, 16 + m : 17 + m], scale=inv_s,
                    )
                    ot = opool.tile([P, BLK], BF16, tag="ot")
                    nc.vector.tensor_sub(ot[:], hb[blk][:, m, :], hct[:])
                    nc.vector.tensor_mul(ot[:], ot[:], zf[:, m, :])
                    nc.vector.tensor_add(ob[:, m, :], ot[:], hct[:])
                    if m % 2 == 1:
                        nc.sync.dma_start(
                            outT[blk, :, m - 1 : m + 1, :], ob[:, m - 1 : m + 1, :]
                        )

    nc.compile()
    return nc


def _pack_feature_major(a: np.ndarray, nchunks: int, dtype) -> np.ndarray:
    # [rows, cols] -> [128, nchunks, cols] with [p, k, c] = a[128k+p, c]
    rows, cols = a.shape
    assert rows == nchunks * P
    return np.ascontiguousarray(
        a.reshape(nchunks, P, cols).transpose(1, 0, 2)
    ).astype(dtype)


def _block_major(a: np.ndarray) -> np.ndarray:
    # [128, K, NB] -> [NBLK, 128, K, BLK]
    p, k, nb = a.shape
    return np.ascontiguousarray(
        a.reshape(p, k, NBLK, BLK).transpose(2, 0, 1, 3)
    )


def _pack_inputs(x, hidden, Wxr, bxr, Whr, Wxz, bxz, Whz, Wxh, bxh, Whh):
    bf = ml_dtypes.bfloat16
    f8 = ml_dtypes.float8_e4m3  # TRN-compatible e4m3 (max 240)
    wxr_p = _pack_feature_major(np.asarray(Wxr, np.float32) * WSCALE, KX, f8)
    wxz_p = _pack_feature_major(np.asarray(Wxz, np.float32) * WSCALE, KX, f8)
    wxh_p = _pack_feature_major(np.asarray(Wxh, np.float32) * WSCALE, KX, bf)
    whr_p = _pack_feature_major(np.asarray(Whr, np.float32) * WSCALE, KH, f8)
    whz_p = _pack_feature_major(np.asarray(Whz, np.float32) * WSCALE, KH, f8)
    whh_p = _pack_feature_major(np.asarray(Whh, np.float32) * WSCALE, KH, f8)
    bias_p = np.ascontiguousarray(
        np.concatenate(
            [
                np.asarray(b, np.float32).reshape(M, P).T
                for b in (bxr, bxz, bxh)
            ],
            axis=1,
        )
    )  # [128, 24]

    x = np.asarray(x, np.float32)
    hidden = np.asarray(hidden, np.float32)

    in_maps = []
    for c in range(NCORES):
        rows = slice(c * NB, (c + 1) * NB)
        xp = _pack_feature_major(x[rows].T, KX, np.float32)   # [128,4,2048]
        hp = _pack_feature_major(hidden[rows].T, KH, np.float32)
        xh = np.concatenate([xp, hp], axis=1)  # [128, 12, 2048]
        rxc_p = np.ascontiguousarray(
            np.concatenate([xp[:, :, 0:BLK].astype(f8), wxr_p, wxz_p], axis=2)
        )
        rhc_p = np.ascontiguousarray(
            np.concatenate([hp[:, :, 0:BLK].astype(f8), whr_p], axis=2)
        )
        bbc_p = np.ascontiguousarray(xh[:, :, 0:BLK].astype(bf))
        in_maps.append(
            {
                "rxc": rxc_p,
                "rhc": rhc_p,
                "whz": whz_p,
                "bbc": bbc_p,
                "xh8": _block_major(xh.astype(f8)),
                "xhb": _block_major(xh.astype(bf)),
                "wxh": wxh_p,
                "whh": whh_p,
                "bias": bias_p,
            }
        )
    return in_maps


def kernel(x, hidden, Wxr, bxr, Whr, Wxz, bxz, Whz, Wxh, bxh, Whh):
    if "nc" not in _CACHE:
        _CACHE["nc"] = _build()
    nc = _CACHE["nc"]

    in_maps = _pack_inputs(
        x, hidden, Wxr, bxr, Whr, Wxz, bxz, Whz, Wxh, bxh, Whh
    )
    res = run_bass_kernel_spmd(nc, in_maps, core_ids=list(range(NCORES)))

    out = np.empty((BATCH, HID), np.float32)
    for c in range(NCORES):
        oT = np.asarray(res.results[c]["outT"], dtype=np.float32)  # [4,128,8,512]
        out[c * NB : (c + 1) * NB] = (
            oT.transpose(2, 1, 0, 3).reshape(HID, NB).T
        )
    return out


# revision 15
# speedup vs baseline: 1.0433x; 1.0433x over previous
"""GRU cell (single timestep) on 8 TRN2 NeuronCores, data-parallel over batch.

Contract: kernel(**inputs) takes FULL numpy inputs (as produced by the
problem's setup_inputs()) and returns the FULL (16384, 1024) float32 output.

Strategy:
  - Shard batch (16384) across 8 cores -> 2048 rows/core; 4 column-blocks
    of 512. Replicate weights.
  - All h-side matmuls, and the r/z-gate x-side, run in fp8 e4m3 with
    perf_mode=DoubleRow (2 k-tiles per MM -> ~1.9x PE rate); the hc-gate
    x-side stays bf16 (fp8 there pushes rel err to ~2e-2; this config
    measures ~1.5e-2 vs the 2e-2 gate). All weights pre-scaled by 1024
    (exact pow2 shift for bf16; lifts fp8 weights out of the subnormal
    range); the activation applies scale=1/1024 before bias+nonlinearity.
  - Activations + elementwise combine in bf16 (DVE 2x packed mode);
    fp32 PSUM accumulation.
  - Head: each dma_start costs ~3.5us of fixed issue/complete latency and
    transfers finish roughly FIFO by issue order, so the t=0 critical set
    is packed into just three fat transfers (same k-chunk structure,
    concatenated along the free dim):
      rxc [128, 4, 2560] fp8 = x8(block0) | 1024*Wxr | 1024*Wxz
      rhc [128, 8, 2560] fp8 = h8(block0) | 1024*Whr | 1024*Whz
      bbc [128, 12, 512] bf16 = xb(block0) | hb(block0)
    A GpSimd fence gated on bbc (the last t=0 transfer) releases the hc
    weights; blocks 1-3 (host-packed fp8+bf16) fetch from the Scalar
    queue after the r activations. 20 dummy matmuls on a memset tile
    warm the PE clock (HAM 1.2->2.4GHz) while the first DMAs land.
"""

import sys

if "/opt/trn_rl_repo" not in sys.path:
    sys.path.insert(0, "/opt/trn_rl_repo")

import numpy as np
import ml_dtypes

import concourse.bass as bass
import concourse.tile as tile
from concourse import bacc, mybir
from concourse.bass_utils import run_bass_kernel_spmd

P = 128
NCORES = 8
BATCH = 16384
NB = BATCH // NCORES          # 2048 rows per core
IN = 512
HID = 1024
KX = IN // P                  # 4
KH = HID // P                 # 8
KA = KX + KH                  # 12 packed k-chunks (x then h)
M = HID // P                  # 8 output-feature chunks
BLK = 512                     # batch columns per compute block
NBLK = NB // BLK              # 4
WSCALE = 1024.0               # pow2 pre-scale on all weights
Z_X_FP8 = True                # z-gate x-side in fp8 DoubleRow
N_WARM = 50                   # dummy matmuls to warm the PE clock

F32 = mybir.dt.float32
BF16 = mybir.dt.bfloat16
FP8 = mybir.dt.float8e4

SIG = mybir.ActivationFunctionType.Sigmoid
TANH = mybir.ActivationFunctionType.Tanh
DR = mybir.MatmulPerfMode.DoubleRow

_CACHE = {}


def _build():
    nc = bacc.Bacc("TRN2", target_bir_lowering=False, debug=False, num_devices=NCORES)

    rxc = nc.dram_tensor("rxc", [P, KX, BLK + 2 * HID], FP8, kind="ExternalInput").ap()
    rhc = nc.dram_tensor("rhc", [P, KH, BLK + HID], FP8, kind="ExternalInput").ap()
    whz = nc.dram_tensor("whz", [P, KH, HID], FP8, kind="ExternalInput").ap()
    bbc = nc.dram_tensor("bbc", [P, KA, BLK], BF16, kind="ExternalInput").ap()
    xh8 = nc.dram_tensor("xh8", [NBLK, P, KA, BLK], FP8, kind="ExternalInput").ap()
    xhb = nc.dram_tensor("xhb", [NBLK, P, KA, BLK], BF16, kind="ExternalInput").ap()
    wxh = nc.dram_tensor("wxh", [P, KX, HID], BF16, kind="ExternalInput").ap()
    whh = nc.dram_tensor("whh", [P, KH, HID], FP8, kind="ExternalInput").ap()
    bias = nc.dram_tensor("bias", [P, 24], F32, kind="ExternalInput").ap()
    outT = nc.dram_tensor("outT", [NBLK, P, M, BLK], BF16, kind="ExternalOutput").ap()

    inv_s = 1.0 / WSCALE

    with tile.TileContext(nc) as tc:
        with (
            tc.tile_pool(name="wpool", bufs=1) as wpool,
            tc.tile_pool(name="x8pool", bufs=3) as x8pool,
            tc.tile_pool(name="h8pool", bufs=3) as h8pool,
            tc.tile_pool(name="xbpool", bufs=3) as xbpool,
            tc.tile_pool(name="hbpool", bufs=3) as hbpool,
            tc.tile_pool(name="rpool", bufs=2) as rpool,
            tc.tile_pool(name="rhpool", bufs=2) as rhpool,
            tc.tile_pool(name="zpool", bufs=2) as zpool,
            tc.tile_pool(name="hcpool", bufs=2) as hcpool,
            tc.tile_pool(name="opool", bufs=4) as opool,
            tc.tile_pool(name="obpool", bufs=2) as obpool,
            tc.tile_pool(name="psum", bufs=8, space=bass.MemorySpace.PSUM) as psum,
        ):
            b_s = wpool.tile([P, 24], F32)
            rxc_s = wpool.tile([P, KX, BLK + 2 * HID], FP8)
            rhc_s = wpool.tile([P, KH, BLK + HID], FP8)
            whz_s = wpool.tile([P, KH, HID], FP8)
            bbc_s = wpool.tile([P, KA, BLK], BF16)
            wxh_s = wpool.tile([P, KX, HID], BF16)
            whh_s = wpool.tile([P, KH, HID], FP8)
            dummy = wpool.tile([P, BLK], BF16)
            fence = wpool.tile([P, 2], BF16)

            wxr_v = rxc_s[:, :, BLK : BLK + HID]
            wxz_v = rxc_s[:, :, BLK + HID : BLK + 2 * HID]
            whr_v = rhc_s[:, :, BLK : BLK + HID]
            whz_v = whz_s

            x8 = [None] * NBLK   # [P, 4, BLK] fp8
            h8 = [None] * NBLK   # [P, 8, BLK] fp8
            xb = [None] * NBLK   # [P, 4, BLK] bf16
            hb = [None] * NBLK   # [P, 8, BLK] bf16
            x8[0] = rxc_s[:, :, 0:BLK]
            h8[0] = rhc_s[:, :, 0:BLK]
            xb[0] = bbc_s[:, 0:KX, :]
            hb[0] = bbc_s[:, KX:KA, :]

            def fetch_block(blk, eng):
                x8[blk] = x8pool.tile([P, KX, BLK], FP8, tag="x8", name="x8")
                eng.dma_start(x8[blk][:], xh8[blk, :, 0:KX, :])
                h8[blk] = h8pool.tile([P, KH, BLK], FP8, tag="h8", name="h8")
                eng.dma_start(h8[blk][:], xh8[blk, :, KX:KA, :])
                hb[blk] = hbpool.tile([P, KH, BLK], BF16, tag="hb", name="hb")
                eng.dma_start(hb[blk][:], xhb[blk, :, KX:KA, :])
                xb[blk] = xbpool.tile([P, KX, BLK], BF16, tag="xb", name="xb")
                eng.dma_start(xb[blk][:], xhb[blk, :, 0:KX, :])

            # PE warmup: dummy matmuls on a memset tile while DMAs land.
            nc.vector.memset(dummy[:], 0.0)
            warm_ps = psum.tile([P, BLK], F32, tag="ps", name="ps")
            for _ in range(N_WARM):
                nc.tensor.matmul(
                    warm_ps[:], dummy[:, 0:P], dummy[:], start=True, stop=True,
                )

            # t=0 critical set: three fat transfers, FIFO priority order.
            nc.sync.dma_start(b_s[:], bias[:])
            nc.sync.dma_start(rxc_s[:], rxc[:])
            nc.sync.dma_start(rhc_s[:], rhc[:])
            nc.sync.dma_start(whz_s[:], whz[:])
            nc.sync.dma_start(bbc_s[:], bbc[:])

            # hc weights, gated on the LAST t=0 transfer so they can't
            # steal bandwidth from the critical set.
            nc.gpsimd.tensor_copy(fence[:], bbc_s[:, 0, 0:2])
            nc.gpsimd.dma_start(wxh_s[:], wxh[:])
            nc.gpsimd.dma_start(whh_s[:], whh[:])

            def x_mms(ps, wx, m, blk, fp8_x):
                mo = bass.ts(m, P)
                if fp8_x:
                    for j in range(KX // 2):
                        nc.tensor.matmul(
                            ps[:], wx[:, 2 * j : 2 * j + 2, mo],
                            x8[blk][:, 2 * j : 2 * j + 2, :],
                            start=(j == 0), stop=False, perf_mode=DR,
                        )
                else:
                    for k in range(KX):
                        nc.tensor.matmul(
                            ps[:], wx[:, k, mo], xb[blk][:, k, :],
                            start=(k == 0), stop=False,
                        )

            def h_mms(ps, wh, m, rhs8):
                mo = bass.ts(m, P)
                for j in range(KH // 2):
                    nc.tensor.matmul(
                        ps[:], wh[:, 2 * j : 2 * j + 2, mo],
                        rhs8[:, 2 * j : 2 * j + 2, :],
                        start=False, stop=(j == KH // 2 - 1),
                        perf_mode=DR,
                    )

            for blk in range(NBLK):
                split = blk == 0  # emit x-groups before h-groups on block 0

                # ---- R phase: r = sigmoid(x@Wxr + bxr + h@Whr); rh8 = fp8(r*h)
                rh8 = rhpool.tile([P, KH, BLK], FP8, tag="rh8")
                ps_r = []
                for m in range(M):
                    ps = psum.tile([P, BLK], F32, tag="ps", name="ps")
                    ps_r.append(ps)
                    x_mms(ps, wxr_v, m, blk, True)
                    if not split:
                        h_mms(ps, whr_v, m, h8[blk])
                if split:
                    for m in range(M):
                        h_mms(ps_r[m], whr_v, m, h8[blk])
                for m in range(M):
                    rt = rpool.tile([P, BLK], BF16, tag="rt")
                    nc.scalar.activation(
                        rt[:], ps_r[m][:], SIG,
                        bias=b_s[:, m : m + 1], scale=inv_s,
                    )
                    eng = nc.vector if m % 2 == 0 else nc.gpsimd
                    eng.tensor_mul(rh8[:, m, :], rt[:], hb[blk][:, m, :])

                if blk == 0:
                    fetch_block(1, nc.scalar)

                # ---- Z phase: z = sigmoid(x@Wxz + bxz + h@Whz)
                zf = zpool.tile([P, M, BLK], BF16, tag="zf")
                ps_z = []
                for m in range(M):
                    ps = psum.tile([P, BLK], F32, tag="ps", name="ps")
                    ps_z.append(ps)
                    x_mms(ps, wxz_v, m, blk, Z_X_FP8)
                    if not split:
                        h_mms(ps, whz_v, m, h8[blk])
                if split:
                    for m in range(M):
                        h_mms(ps_z[m], whz_v, m, h8[blk])
                for m in range(M):
                    nc.scalar.activation(
                        zf[:, m, :], ps_z[m][:], SIG,
                        bias=b_s[:, 8 + m : 9 + m], scale=inv_s,
                    )

                if blk + 2 < NBLK:
                    fetch_block(blk + 2, nc.scalar)

                # ---- HC phase: hc = tanh(x@Wxh + bxh + rh@Whh)
                #      out = hc + z*(h - hc)
                ob = obpool.tile([P, M, BLK], BF16, tag="ob")
                for m in range(M):
                    ps = psum.tile([P, BLK], F32, tag="ps", name="ps")
                    x_mms(ps, wxh_s, m, blk, False)
                    h_mms(ps, whh_s, m, rh8)
                    hct = hcpool.tile([P, BLK], BF16, tag="hct")
                    nc.scalar.activation(
                        hct[:], ps[:], TANH,
                        bias=b_s[:, 16 + m : 17 + m], scale=inv_s,
                    )
                    ot = opool.tile([P, BLK], BF16, tag="ot")
                    nc.vector.tensor_sub(ot[:], hb[blk][:, m, :], hct[:])
                    nc.vector.tensor_mul(ot[:], ot[:], zf[:, m, :])
                    nc.vector.tensor_add(ob[:, m, :], ot[:], hct[:])
                    if m % 2 == 1:
                        nc.sync.dma_start(
                            outT[blk, :, m - 1 : m + 1, :], ob[:, m - 1 : m + 1, :]
                        )

    nc.compile()
    return nc


def _pack_feature_major(a: np.ndarray, nchunks: int, dtype) -> np.ndarray:
    # [rows, cols] -> [128, nchunks, cols] with [p, k, c] = a[128k+p, c]
    rows, cols = a.shape
    assert rows == nchunks * P
    return np.ascontiguousarray(
        a.reshape(nchunks, P, cols).transpose(1, 0, 2)
    ).astype(dtype)


def _block_major(a: np.ndarray) -> np.ndarray:
    # [128, K, NB] -> [NBLK, 128, K, BLK]
    p, k, nb = a.shape
    return np.ascontiguousarray(
        a.reshape(p, k, NBLK, BLK).transpose(2, 0, 1, 3)
    )


def _pack_inputs(x, hidden, Wxr, bxr, Whr, Wxz, bxz, Whz, Wxh, bxh, Whh):
    bf = ml_dtypes.bfloat16
    f8 = ml_dtypes.float8_e4m3  # TRN-compatible e4m3 (max 240)
    wxr_p = _pack_feature_major(np.asarray(Wxr, np.float32) * WSCALE, KX, f8)
    wxz_p = _pack_feature_major(np.asarray(Wxz, np.float32) * WSCALE, KX, f8)
    wxh_p = _pack_feature_major(np.asarray(Wxh, np.float32) * WSCALE, KX, bf)
    whr_p = _pack_feature_major(np.asarray(Whr, np.float32) * WSCALE, KH, f8)
    whz_p = _pack_feature_major(np.asarray(Whz, np.float32) * WSCALE, KH, f8)
    whh_p = _pack_feature_major(np.asarray(Whh, np.float32) * WSCALE, KH, f8)
    bias_p = np.ascontiguousarray(
        np.concatenate(
            [
                np.asarray(b, np.float32).reshape(M, P).T
                for b in (bxr, bxz, bxh)
            ],
            axis=1,
        )
    )  # [128, 24]

    x = np.asarray(x, np.float32)
    hidden = np.asarray(hidden, np.float32)

    in_maps = []
    for c in range(NCORES):
        rows = slice(c * NB, (c + 1) * NB)
        xp = _pack_feature_major(x[rows].T, KX, np.float32)   # [128,4,2048]
        hp = _pack_feature_major(hidden[rows].T, KH, np.float32)
        xh = np.concatenate([xp, hp], axis=1)  # [128, 12, 2048]
        rxc_p = np.ascontiguousarray(
            np.concatenate([xp[:, :, 0:BLK].astype(f8), wxr_p, wxz_p], axis=2)
        )
        rhc_p = np.ascontiguousarray(
            np.concatenate([hp[:, :, 0:BLK].astype(f8), whr_p], axis=2)
        )
        bbc_p = np.ascontiguousarray(xh[:, :, 0:BLK].astype(bf))
        in_maps.append(
            {
                "rxc": rxc_p,
                "rhc": rhc_p,
                "whz": whz_p,
                "bbc": bbc_p,
                "xh8": _block_major(xh.astype(f8)),
                "xhb": _block_major(xh.astype(bf)),
                "wxh": wxh_p,
                "whh": whh_p,
                "bias": bias_p,
            }
        )
    return in_maps


def kernel(x, hidden, Wxr, bxr, Whr, Wxz, bxz, Whz, Wxh, bxh, Whh):
    if "nc" not in _CACHE:
        _CACHE["nc"] = _build()
    nc = _CACHE["nc"]

    in_maps = _pack_inputs(
        x, hidden, Wxr, bxr, Whr, Wxz, bxz, Whz, Wxh, bxh, Whh
    )
    res = run_bass_kernel_spmd(nc, in_maps, core_ids=list(range(NCORES)))

    out = np.empty((BATCH, HID), np.float32)
    for c in range(NCORES):
        oT = np.asarray(res.results[c]["outT"], dtype=np.float32)  # [4,128,8,512]
        out[c * NB : (c + 1) * NB] = (
            oT.transpose(2, 1, 0, 3).reshape(HID, NB).T
        )
    return out
